# revision 20
# baseline (speedup 1.0000x reference)
"""Trainium2 Bass kernel for DigitConvolutionalModel.

Model: x[B,784] -> reshape 28x28 -> 3x3 valid conv (weights conv_w) ->
[B,676] -> Linear(676,100)+relu -> Linear(100,10)+relu -> Linear(10,10).

The conv is linear, so it folds into the first Linear: W1f = C @ w1 where
C[784,676] is the conv unfold matrix. The whole model becomes a 3-layer MLP
784 -> 100 -> 10 -> 10 with relu between layers.

Sharding: pure data parallel, batch split across 8 cores (8192 rows each).

Precision: matmuls in bf16, accumulation in fp32 PSUM, biases + output fp32.

Key structural choices (from several rounds of NTFF trace analysis):
- ALL loads ride ONE HWDGE FIFO (the sync queue): weights blob, pair-0 x,
  packed tails, then the remaining 7 pair loads. Measured: while the sync
  ring moves 12KB/partition descriptors it starves every other queue
  (scalar-ring weights arrived ~6us late, gpsimd tail transfers stalled
  mid-stream and collapsed the main stream to ~66GB/s), so nothing
  latency-critical may ride another queue concurrently.
- x features 0..767 stream as 8 uniform 2-supertile loads ([128, 2, 6, TN],
  12KB/partition descriptors reach the ~420GB/s line rate; 112-partition
  and sub-6KB-descriptor layouts measured at 100-280GB/s).
- tail features 768..783 are packed 2 supertiles per 128-partition transfer
  at partition bases {0,64} (bass allows matmul row bases 0/32/64 only).
  The tail matmuls use those bases directly against a 2x-replicated w1t in
  the blob. This keeps the DMA engines balanced: a plain [16, ...] transfer
  uses only 4 of 16.
- per-pair [*, 2, TN] PSUM tiles spanning 2 banks: one ACT activation per
  layer per pair. PSUM budget: L1 2x2 + L2 2 + L3 2 = 8 banks.
- warmup matmuls bridge PE boot (~6.9us) to first-data (~14us): the HAM
  clock gate re-throttles after ~3.4us of PE idle, and a cold matmul runs
  at 1.2GHz instead of 2.4.
- stores ride gpsimd (SWDGE): they starve while the load stream runs and
  flush at stream end, which is harmless because every pair has its own
  output buffer (no reuse waits). Last pair stores on the by-then-idle
  sync queue.
- last pair runs as two parallel half-chains (ACT half / DVE half) with
  per-half stores to shorten the serial L1->relu->L2->relu->L3->add->store
  drain after the final matmul.
"""

import numpy as np
import ml_dtypes

import concourse.bacc as bacc
import concourse.tile as tile
from concourse.tile import add_dep_helper
from concourse import mybir
from concourse.bass_utils import run_bass_kernel_spmd

N_CORES = 8
B = 65536
BC = B // N_CORES  # 8192 rows per core
TN = 512           # batch columns per supertile
NT = BC // TN      # 16 supertiles per core
NKC = 6            # full 128-feature chunks (0..767)
KT = 16            # tail features (768..783)
NTB = NT // 2      # tail blocks (2 supertiles each, bases 0/64)
NF = 784
H1 = 100
HO = 10
F32 = mybir.dt.float32
BF16 = mybir.dt.bfloat16
NP_BF16 = ml_dtypes.bfloat16

NPAIR = NT // 2

# packed weight blob column layout (bf16 columns)
_C_W1M = 0                      # [128, 600]  w1 main chunks
_C_W1T = 600                    # [128, 100]  w1t replicated at bases 0/32/64/96
_C_W2 = 700                    # [100, 10]   w2
_C_W3 = 710                    # [10, 10]    w3
_C_B1 = 720                    # [100, 2]    b1 as f32 byte-pairs
_C_B2 = 722                    # [10, 2]     b2
_C_B3 = 724                    # [10, 2]     b3
WBW = 726

N_WARMUP = 22
WUN = 512  # warmup matmul free dim


def _build_nc():
    nc = bacc.Bacc(None, target_bir_lowering=False)

    # feature-major, partition-major-first; per pair 13 slots of [128, TN]:
    # 0-5 = supertile-a main chunks, 6 = packed tails (a at rows 0..15, b
    # at rows 64..79, rest zero), 7-12 = b main chunks. Loaded as TWO
    # transfers per pair (slots 0-6 and 7-12): finer completion
    # granularity keeps every PE data-wait under the ~3.4us HAM
    # re-throttle threshold and brings first-data ~2us earlier.
    xt = nc.dram_tensor("xt", [128, NPAIR, 13, TN], BF16,
                        kind="ExternalInput")
    wblob = nc.dram_tensor("wblob", [128, WBW], BF16, kind="ExternalInput")
    yt = nc.dram_tensor("yt", [HO, BC], F32, kind="ExternalOutput")

    relu = mybir.ActivationFunctionType.Relu
    ident = mybir.ActivationFunctionType.Identity

    with tile.TileContext(nc) as tc:
        with (
            tc.tile_pool(name="const", bufs=1) as cpool,
            tc.tile_pool(name="io", bufs=1) as iopool,
            tc.tile_pool(name="act", bufs=3) as apool,
            tc.tile_pool(name="out", bufs=1) as opool,
            tc.tile_pool(name="ps1", bufs=2, space="PSUM") as ps1,
            tc.tile_pool(name="ps2", bufs=1, space="PSUM") as ps2,
            tc.tile_pool(name="ps3", bufs=1, space="PSUM") as ps3,
        ):
            # sync FIFO order: weights, then 16 half-pair loads
            wb_s = cpool.tile([128, WBW], BF16, tag="wb")
            nc.sync.dma_start(wb_s[:], wblob[:])
            xg = []
            for p in range(NPAIR):
                ta = iopool.tile([128, 7, TN], BF16, tag=f"xa{p}")
                nc.sync.dma_start(ta[:], xt[:, p, 0:7])
                tb = iopool.tile([128, 6, TN], BF16, tag=f"xb{p}")
                nc.sync.dma_start(tb[:], xt[:, p, 7:13])
                xg.append((ta, tb))

            w2_ap = wb_s[0:H1, _C_W2:_C_W2 + HO]
            w3_ap = wb_s[0:HO, _C_W3:_C_W3 + HO]
            b1_ap = wb_s[0:H1, _C_B1:_C_B1 + 2].bitcast(F32)
            b2_ap = wb_s[0:HO, _C_B2:_C_B2 + 2].bitcast(F32)
            b3_ap = wb_s[0:HO, _C_B3:_C_B3 + 2].bitcast(F32)

            def w1_ap(k):
                return wb_s[:, _C_W1M + k * H1:_C_W1M + (k + 1) * H1]

            def w1t_ap(j):
                base = 64 * j
                return wb_s[base:base + KT, _C_W1T:_C_W1T + H1]

            # All matmuls chained with same-engine ordering deps so the PE
            # executes them in emission order (required for ldweights=False
            # weight reuse from the previous matmul).
            prev_mm = [None]

            def mm(out_ap, lhsT_ap, rhs_ap, start, stop, ldw=True):
                m = nc.tensor.matmul(out_ap, lhsT_ap, rhs_ap,
                                     start=start, stop=stop)
                if not ldw:
                    m.ins.ldweights = False
                if prev_mm[0] is not None:
                    add_dep_helper(m.ins, prev_mm[0], sync=False,
                                   reason="pe program order")
                prev_mm[0] = m.ins
                return m

            # Warmup: dummy matmuls bridge the PE-boot -> first-data window
            # so the HAM clock is warm for the real stream. They multiply
            # garbage (wsc is memset AFTER emission: WAR, not RAW, so they
            # start at the engines-go barrier).
            wsc = cpool.tile([128, 2 * TN], BF16, tag="wsc")
            wp = ps1.tile([H1, 2, TN], F32, tag="p1")
            mm(wp[:, 0, :], wsc[:, 0:H1], wsc[:, 0:WUN],
               start=True, stop=True)
            for _ in range(N_WARMUP - 1):
                mm(wp[:, 0, :], wsc[:, 0:H1], wsc[:, 0:WUN],
                   start=True, stop=True, ldw=False)
            nc.vector.memset(wsc[:], 0.0)

            h1s: dict[int, object] = {}
            h2s: dict[int, object] = {}

            def emit_l1(p):
                last = p == NPAIR - 1
                xa, xb = xg[p]
                p1 = ps1.tile([H1, 2, TN], F32, tag="p1")
                # sequential supertile emission: each half waits only its
                # own (half-pair) load
                h1e = None
                for k in range(NKC):
                    mm(p1[:, 0, :], w1_ap(k), xa[:, k, :],
                       start=(k == 0), stop=False)
                mm(p1[:, 0, :], w1t_ap(0), xa[0:KT, 6, :],
                   start=False, stop=True)
                if last:
                    # a's relu overlaps b's L1 matmuls
                    h1e = apool.tile([H1, 2, TN], BF16, tag="h1")
                    nc.scalar.activation(h1e[:, 0, :], p1[:, 0, :], relu,
                                         bias=b1_ap)
                for k in range(NKC):
                    mm(p1[:, 1, :], w1_ap(k), xb[:, k, :],
                       start=(k == 0), stop=False)
                mm(p1[:, 1, :], w1t_ap(1), xa[64:64 + KT, 6, :],
                   start=False, stop=True)
                if last:
                    nc.vector.scalar_tensor_tensor(
                        h1e[:, 1, :], p1[:, 1, :], b1_ap, wsc[0:H1, 0:TN],
                        op0=mybir.AluOpType.add, op1=mybir.AluOpType.max)
                    h1 = h1e
                else:
                    h1 = apool.tile([H1, 2, TN], BF16, tag="h1")
                    nc.scalar.activation(h1[:], p1[:], relu, bias=b1_ap)
                h1s[p] = h1

            def emit_l2(p):
                h1 = h1s.pop(p)
                p2 = ps2.tile([HO, 2, TN], F32, tag="p2")
                mm(p2[:, 0, :], w2_ap, h1[:, 0, :], start=True, stop=True)
                mm(p2[:, 1, :], w2_ap, h1[:, 1, :], start=True, stop=True,
                   ldw=False)
                h2 = apool.tile([HO, 2, TN], BF16, tag="h2")
                if p == NPAIR - 1:
                    nc.scalar.activation(h2[:, 0, :], p2[:, 0, :], relu,
                                         bias=b2_ap)
                    nc.vector.scalar_tensor_tensor(
                        h2[:, 1, :], p2[:, 1, :], b2_ap, wsc[0:HO, 0:TN],
                        op0=mybir.AluOpType.add, op1=mybir.AluOpType.max)
                else:
                    nc.scalar.activation(h2[:], p2[:], relu, bias=b2_ap)
                h2s[p] = h2

            def emit_l3(p):
                h2 = h2s.pop(p)
                p3 = ps3.tile([HO, 2, TN], F32, tag="p3")
                mm(p3[:, 0, :], w3_ap, h2[:, 0, :], start=True, stop=True)
                mm(p3[:, 1, :], w3_ap, h2[:, 1, :], start=True, stop=True,
                   ldw=False)
                # per-pair output buffer: stores may flush late (SWDGE
                # starves under the HWDGE load stream) without stalling
                # anything
                ot = opool.tile([HO, 2, TN], F32, tag=f"ot{p}")
                dst = yt[:, 2 * p * TN:(2 * p + 2) * TN]
                if p == NPAIR - 1:
                    # split halves: a finishes on ACT (Identity+bias), b on
                    # DVE; stores issue as each half lands
                    nc.scalar.activation(ot[:, 0, :], p3[:, 0, :], ident,
                                         bias=b3_ap)
                    nc.sync.dma_start(yt[:, 2 * p * TN:(2 * p + 1) * TN],
                                      ot[:, 0, :])
                    nc.vector.scalar_tensor_tensor(
                        ot[:, 1, :], p3[:, 1, :], b3_ap, wsc[0:HO, 0:TN],
                        op0=mybir.AluOpType.add, op1=mybir.AluOpType.add)
                    nc.sync.dma_start(yt[:, (2 * p + 1) * TN:(2 * p + 2) * TN],
                                      ot[:, 1, :])
                else:
                    nc.vector.scalar_tensor_tensor(
                        ot[:], p3[:], b3_ap, wsc[0:HO, :],
                        op0=mybir.AluOpType.add, op1=mybir.AluOpType.add)
                    if p == NPAIR - 2:
                        nc.sync.dma_start(dst, ot[:])
                    else:
                        nc.gpsimd.dma_start(dst, ot[:])

            # 3-stage software pipeline: L1(p), L2(p-1), L3(p-2)
            for p in range(NPAIR + 2):
                if p < NPAIR:
                    emit_l1(p)
                if 1 <= p <= NPAIR:
                    emit_l2(p - 1)
                if p >= 2:
                    emit_l3(p - 2)

    nc.compile()
    return nc


def _fold_conv_into_w1(conv_w: np.ndarray, w1: np.ndarray) -> np.ndarray:
    """W1f[784,100] such that x @ W1f == conv(x).reshape(B,676) @ w1."""
    c = np.zeros((NF, 26 * 26), dtype=np.float64)
    for di in range(3):
        for dj in range(3):
            ii, jj = np.meshgrid(np.arange(26), np.arange(26), indexing="ij")
            src = (ii + di) * 28 + (jj + dj)
            dst = ii * 26 + jj
            c[src.ravel(), dst.ravel()] += np.float64(conv_w[di, dj])
    return (c @ w1.astype(np.float64)).astype(np.float32)


def _prep_in_maps(x, conv_w, w1, b1, w2, b2, w3, b3):
    x = np.asarray(x, dtype=np.float32)
    conv_w = np.asarray(conv_w, dtype=np.float32)
    w1 = np.asarray(w1, dtype=np.float32)
    b1 = np.asarray(b1, dtype=np.float32)
    w2 = np.asarray(w2, dtype=np.float32)
    b2 = np.asarray(b2, dtype=np.float32)
    w3 = np.asarray(w3, dtype=np.float32)
    b3 = np.asarray(b3, dtype=np.float32)

    w1f = _fold_conv_into_w1(conv_w, w1)  # [784, 100]
    # main chunks: feature f = k*128 + p -> [128, 600]
    w1m = np.ascontiguousarray(
        w1f[: 128 * NKC].reshape(NKC, 128, H1).transpose(1, 0, 2)
    ).astype(NP_BF16).reshape(128, NKC * H1)
    w1t = w1f[128 * NKC:].astype(NP_BF16)  # [16, 100]

    blob = np.zeros((128, WBW), np.uint16)
    blob[:, _C_W1M:_C_W1M + NKC * H1] = w1m.view(np.uint16)
    for c in range(2):
        blob[64 * c:64 * c + KT, _C_W1T:_C_W1T + H1] = w1t.view(np.uint16)
    blob[0:H1, _C_W2:_C_W2 + HO] = w2.astype(NP_BF16).view(np.uint16)
    blob[0:HO, _C_W3:_C_W3 + HO] = w3.astype(NP_BF16).view(np.uint16)
    blob[0:H1, _C_B1:_C_B1 + 2] = b1.reshape(H1, 1).view(np.uint16)
    blob[0:HO, _C_B2:_C_B2 + 2] = b2.reshape(HO, 1).view(np.uint16)
    blob[0:HO, _C_B3:_C_B3 + 2] = b3.reshape(HO, 1).view(np.uint16)
    shared = {"wblob": blob.view(NP_BF16)}

    xb = x.astype(NP_BF16)  # cast once, full batch
    in_maps = []
    for core in range(N_CORES):
        xc = xb[core * BC:(core + 1) * BC]  # [8192, 784] bf16
        xct = xc.reshape(NT, TN, NF).transpose(0, 2, 1)  # [NT, NF, TN]
        mains = xct[:, : 128 * NKC].reshape(NT, NKC, 128, TN)
        tails = xct[:, 128 * NKC:]  # [NT, 16, TN]
        xt_all = np.zeros((128, NPAIR, 13, TN), dtype=NP_BF16)
        xt_all[:, :, 0:NKC] = mains[0::2].transpose(2, 0, 1, 3)
        xt_all[:, :, 7:7 + NKC] = mains[1::2].transpose(2, 0, 1, 3)
        xt_all[0:KT, :, NKC] = tails[0::2].transpose(1, 0, 2)
        xt_all[64:64 + KT, :, NKC] = tails[1::2].transpose(1, 0, 2)
        in_maps.append({"xt": xt_all, **shared})
    return in_maps


_NC = None


def _get_nc():
    global _NC
    if _NC is None:
        _NC = _build_nc()
    return _NC


def kernel(x, conv_w, w1, b1, w2, b2, w3, b3):
    in_maps = _prep_in_maps(x, conv_w, w1, b1, w2, b2, w3, b3)
    nc = _get_nc()
    res = run_bass_kernel_spmd(nc, in_maps, core_ids=list(range(N_CORES)))
    out = np.empty((B, HO), dtype=np.float32)
    for i in range(N_CORES):
        out[i * BC:(i + 1) * BC] = res.results[i]["yt"].T
    return out


if __name__ == "__main__":
    rng = np.random.default_rng(0)
    inputs = {
        "x": rng.standard_normal((B, NF), dtype=np.float32),
        "conv_w": np.ones((3, 3), dtype=np.float32),
        "w1": (rng.standard_normal((676, H1)) * 0.04).astype(np.float32),
        "b1": np.zeros(H1, dtype=np.float32),
        "w2": (rng.standard_normal((H1, HO)) * 0.1).astype(np.float32),
        "b2": np.zeros(HO, dtype=np.float32),
        "w3": (rng.standard_normal((HO, HO)) * 0.3).astype(np.float32),
        "b3": np.zeros(HO, dtype=np.float32),
    }
    out = kernel(**inputs)
    print(out.shape, out.dtype)


# revision 22
# speedup vs baseline: 1.0024x; 1.0024x over previous
"""Trainium2 Bass kernel for DigitConvolutionalModel.

Model: x[B,784] -> reshape 28x28 -> 3x3 valid conv (weights conv_w) ->
[B,676] -> Linear(676,100)+relu -> Linear(100,10)+relu -> Linear(10,10).

The conv is linear, so it folds into the first Linear: W1f = C @ w1 where
C[784,676] is the conv unfold matrix. The whole model becomes a 3-layer MLP
784 -> 100 -> 10 -> 10 with relu between layers.

Sharding: pure data parallel, batch split across 8 cores (8192 rows each).

Precision: matmuls in bf16, accumulation in fp32 PSUM, biases + output fp32.

Key structural choices (from several rounds of NTFF trace analysis):
- ALL loads ride ONE HWDGE FIFO (the sync queue): weights blob, pair-0 x,
  packed tails, then the remaining 7 pair loads. Measured: while the sync
  ring moves 12KB/partition descriptors it starves every other queue
  (scalar-ring weights arrived ~6us late, gpsimd tail transfers stalled
  mid-stream and collapsed the main stream to ~66GB/s), so nothing
  latency-critical may ride another queue concurrently.
- x features 0..767 stream as 8 uniform 2-supertile loads ([128, 2, 6, TN],
  12KB/partition descriptors reach the ~420GB/s line rate; 112-partition
  and sub-6KB-descriptor layouts measured at 100-280GB/s).
- tail features 768..783 are packed 2 supertiles per 128-partition transfer
  at partition bases {0,64} (bass allows matmul row bases 0/32/64 only).
  The tail matmuls use those bases directly against a 2x-replicated w1t in
  the blob. This keeps the DMA engines balanced: a plain [16, ...] transfer
  uses only 4 of 16.
- per-pair [*, 2, TN] PSUM tiles spanning 2 banks: one ACT activation per
  layer per pair. PSUM budget: L1 2x2 + L2 2 + L3 2 = 8 banks.
- warmup matmuls bridge PE boot (~6.9us) to first-data (~14us): the HAM
  clock gate re-throttles after ~3.4us of PE idle, and a cold matmul runs
  at 1.2GHz instead of 2.4.
- stores ride gpsimd (SWDGE): they starve while the load stream runs and
  flush at stream end, which is harmless because every pair has its own
  output buffer (no reuse waits). Last pair stores on the by-then-idle
  sync queue.
- last pair runs as two parallel half-chains (ACT half / DVE half) with
  per-half stores to shorten the serial L1->relu->L2->relu->L3->add->store
  drain after the final matmul.
"""

import numpy as np
import ml_dtypes

import concourse.bacc as bacc
import concourse.tile as tile
from concourse.tile import add_dep_helper
from concourse import mybir
from concourse.bass_utils import run_bass_kernel_spmd

N_CORES = 8
B = 65536
BC = B // N_CORES  # 8192 rows per core
TN = 512           # batch columns per supertile
NT = BC // TN      # 16 supertiles per core
NKC = 6            # full 128-feature chunks (0..767)
KT = 16            # tail features (768..783)
NTB = NT // 2      # tail blocks (2 supertiles each, bases 0/64)
NF = 784
H1 = 100
HO = 10
F32 = mybir.dt.float32
BF16 = mybir.dt.bfloat16
NP_BF16 = ml_dtypes.bfloat16

NPAIR = NT // 2

# packed weight blob column layout (bf16 columns)
_C_W1M = 0                      # [128, 600]  w1 main chunks
_C_W1T = 600                    # [128, 100]  w1t replicated at bases 0/32/64/96
_C_W2 = 700                    # [100, 10]   w2
_C_W3 = 710                    # [10, 10]    w3
_C_B1 = 720                    # [100, 2]    b1 as f32 byte-pairs
_C_B2 = 722                    # [10, 2]     b2
_C_B3 = 724                    # [10, 2]     b3
WBW = 726

N_WARMUP = 22
WUN = 512  # warmup matmul free dim


def _build_nc():
    nc = bacc.Bacc(None, target_bir_lowering=False)

    # feature-major, partition-major-first; per pair 13 slots of [128, TN]:
    # 0-5 = supertile-a main chunks, 6 = packed tails (a at rows 0..15, b
    # at rows 64..79, rest zero), 7-12 = b main chunks. Loaded as TWO
    # transfers per pair (slots 0-6 and 7-12): finer completion
    # granularity keeps every PE data-wait under the ~3.4us HAM
    # re-throttle threshold and brings first-data ~2us earlier.
    xt = nc.dram_tensor("xt", [128, NPAIR, 13, TN], BF16,
                        kind="ExternalInput")
    wblob = nc.dram_tensor("wblob", [128, WBW], BF16, kind="ExternalInput")
    yt = nc.dram_tensor("yt", [HO, BC], F32, kind="ExternalOutput")

    relu = mybir.ActivationFunctionType.Relu
    ident = mybir.ActivationFunctionType.Identity

    with tile.TileContext(nc) as tc:
        with (
            tc.tile_pool(name="const", bufs=1) as cpool,
            tc.tile_pool(name="io", bufs=1) as iopool,
            tc.tile_pool(name="act", bufs=3) as apool,
            tc.tile_pool(name="out", bufs=1) as opool,
            tc.tile_pool(name="ps1", bufs=2, space="PSUM") as ps1,
            tc.tile_pool(name="ps2", bufs=1, space="PSUM") as ps2,
            tc.tile_pool(name="ps3", bufs=1, space="PSUM") as ps3,
        ):
            # sync FIFO order: weights, then 16 half-pair loads
            wb_s = cpool.tile([128, WBW], BF16, tag="wb")
            nc.sync.dma_start(wb_s[:], wblob[:])
            xg = []
            for p in range(NPAIR):
                ta = iopool.tile([128, 7, TN], BF16, tag=f"xa{p}")
                nc.sync.dma_start(ta[:], xt[:, p, 0:7])
                tb = iopool.tile([128, 6, TN], BF16, tag=f"xb{p}")
                nc.sync.dma_start(tb[:], xt[:, p, 7:13])
                xg.append((ta, tb))

            w2_ap = wb_s[0:H1, _C_W2:_C_W2 + HO]
            w3_ap = wb_s[0:HO, _C_W3:_C_W3 + HO]
            b1_ap = wb_s[0:H1, _C_B1:_C_B1 + 2].bitcast(F32)
            b2_ap = wb_s[0:HO, _C_B2:_C_B2 + 2].bitcast(F32)
            b3_ap = wb_s[0:HO, _C_B3:_C_B3 + 2].bitcast(F32)

            def w1_ap(k):
                return wb_s[:, _C_W1M + k * H1:_C_W1M + (k + 1) * H1]

            def w1t_ap(j):
                base = 64 * j
                return wb_s[base:base + KT, _C_W1T:_C_W1T + H1]

            # All matmuls chained with same-engine ordering deps so the PE
            # executes them in emission order (required for ldweights=False
            # weight reuse from the previous matmul).
            prev_mm = [None]

            def mm(out_ap, lhsT_ap, rhs_ap, start, stop, ldw=True):
                m = nc.tensor.matmul(out_ap, lhsT_ap, rhs_ap,
                                     start=start, stop=stop)
                if not ldw:
                    m.ins.ldweights = False
                if prev_mm[0] is not None:
                    add_dep_helper(m.ins, prev_mm[0], sync=False,
                                   reason="pe program order")
                prev_mm[0] = m.ins
                return m

            # Warmup: dummy matmuls bridge the PE-boot -> first-data window
            # so the HAM clock is warm for the real stream. They multiply
            # garbage (wsc is memset AFTER emission: WAR, not RAW, so they
            # start at the engines-go barrier).
            wsc = cpool.tile([128, 2 * TN], BF16, tag="wsc")
            wp = ps1.tile([H1, 2, TN], F32, tag="p1")
            mm(wp[:, 0, :], wsc[:, 0:H1], wsc[:, 0:WUN],
               start=True, stop=True)
            for _ in range(N_WARMUP - 1):
                mm(wp[:, 0, :], wsc[:, 0:H1], wsc[:, 0:WUN],
                   start=True, stop=True, ldw=False)
            nc.vector.memset(wsc[:], 0.0)

            h1s: dict[int, object] = {}
            h2s: dict[int, object] = {}

            def emit_l1(p):
                last = p == NPAIR - 1
                xa, xb = xg[p]
                p1 = ps1.tile([H1, 2, TN], F32, tag="p1")
                # sequential supertile emission: each half waits only its
                # own (half-pair) load
                for k in range(NKC):
                    mm(p1[:, 0, :], w1_ap(k), xa[:, k, :],
                       start=(k == 0), stop=False)
                mm(p1[:, 0, :], w1t_ap(0), xa[0:KT, 6, :],
                   start=False, stop=True)
                for k in range(NKC):
                    mm(p1[:, 1, :], w1_ap(k), xb[:, k, :],
                       start=(k == 0), stop=False)
                mm(p1[:, 1, :], w1t_ap(1), xa[64:64 + KT, 6, :],
                   start=False, stop=True)
                h1 = apool.tile([H1, 2, TN], BF16, tag="h1")
                if last:
                    # last pair: two parallel half-chains (ACT / DVE) to
                    # shorten the serial drain after the final L1 matmul
                    nc.scalar.activation(h1[:, 0, :], p1[:, 0, :], relu,
                                         bias=b1_ap)
                    nc.vector.scalar_tensor_tensor(
                        h1[:, 1, :], p1[:, 1, :], b1_ap, wsc[0:H1, 0:TN],
                        op0=mybir.AluOpType.add, op1=mybir.AluOpType.max)
                else:
                    nc.scalar.activation(h1[:], p1[:], relu, bias=b1_ap)
                h1s[p] = h1

            def emit_l2(p):
                h1 = h1s.pop(p)
                p2 = ps2.tile([HO, 2, TN], F32, tag="p2")
                mm(p2[:, 0, :], w2_ap, h1[:, 0, :], start=True, stop=True)
                mm(p2[:, 1, :], w2_ap, h1[:, 1, :], start=True, stop=True,
                   ldw=False)
                h2 = apool.tile([HO, 2, TN], BF16, tag="h2")
                if p == NPAIR - 1:
                    nc.scalar.activation(h2[:, 0, :], p2[:, 0, :], relu,
                                         bias=b2_ap)
                    nc.vector.scalar_tensor_tensor(
                        h2[:, 1, :], p2[:, 1, :], b2_ap, wsc[0:HO, 0:TN],
                        op0=mybir.AluOpType.add, op1=mybir.AluOpType.max)
                else:
                    nc.scalar.activation(h2[:], p2[:], relu, bias=b2_ap)
                h2s[p] = h2

            def emit_l3(p):
                h2 = h2s.pop(p)
                if p == NPAIR - 1:
                    # borrow a free ps1 slot (pair N-2's L1 bank, long
                    # since read): ps3's single slot would stall these
                    # matmuls ~2.3us waiting for ot(N-2)'s read
                    p3f = ps1.tile([H1, 2, TN], F32, tag="p1")
                    p3 = p3f[0:HO]
                else:
                    p3 = ps3.tile([HO, 2, TN], F32, tag="p3")
                mm(p3[:, 0, :], w3_ap, h2[:, 0, :], start=True, stop=True)
                mm(p3[:, 1, :], w3_ap, h2[:, 1, :], start=True, stop=True,
                   ldw=False)
                # per-pair output buffer: stores may flush late (SWDGE
                # starves under the HWDGE load stream) without stalling
                # anything
                ot = opool.tile([HO, 2, TN], F32, tag=f"ot{p}")
                dst = yt[:, 2 * p * TN:(2 * p + 2) * TN]
                if p == NPAIR - 1:
                    # split halves: a finishes on ACT (Identity+bias), b on
                    # DVE; stores issue as each half lands
                    nc.scalar.activation(ot[:, 0, :], p3[:, 0, :], ident,
                                         bias=b3_ap)
                    nc.sync.dma_start(yt[:, 2 * p * TN:(2 * p + 1) * TN],
                                      ot[:, 0, :])
                    nc.vector.scalar_tensor_tensor(
                        ot[:, 1, :], p3[:, 1, :], b3_ap, wsc[0:HO, 0:TN],
                        op0=mybir.AluOpType.add, op1=mybir.AluOpType.add)
                    nc.sync.dma_start(yt[:, (2 * p + 1) * TN:(2 * p + 2) * TN],
                                      ot[:, 1, :])
                elif p >= NPAIR - 3:
                    # pairs N-3/N-2: output via ACT Identity so the DVE
                    # queue is clear for the final pair's half-chain
                    nc.scalar.activation(ot[:], p3[:], ident, bias=b3_ap)
                    if p == NPAIR - 2:
                        nc.sync.dma_start(dst, ot[:])
                    else:
                        nc.gpsimd.dma_start(dst, ot[:])
                else:
                    nc.vector.scalar_tensor_tensor(
                        ot[:], p3[:], b3_ap, wsc[0:HO, :],
                        op0=mybir.AluOpType.add, op1=mybir.AluOpType.add)
                    nc.gpsimd.dma_start(dst, ot[:])

            # 3-stage software pipeline: L1(p), L2(p-1), L3(p-2)
            for p in range(NPAIR + 2):
                if p < NPAIR:
                    emit_l1(p)
                if 1 <= p <= NPAIR:
                    emit_l2(p - 1)
                if p >= 2:
                    emit_l3(p - 2)

    nc.compile()
    return nc


def _fold_conv_into_w1(conv_w: np.ndarray, w1: np.ndarray) -> np.ndarray:
    """W1f[784,100] such that x @ W1f == conv(x).reshape(B,676) @ w1."""
    c = np.zeros((NF, 26 * 26), dtype=np.float64)
    for di in range(3):
        for dj in range(3):
            ii, jj = np.meshgrid(np.arange(26), np.arange(26), indexing="ij")
            src = (ii + di) * 28 + (jj + dj)
            dst = ii * 26 + jj
            c[src.ravel(), dst.ravel()] += np.float64(conv_w[di, dj])
    return (c @ w1.astype(np.float64)).astype(np.float32)


def _prep_in_maps(x, conv_w, w1, b1, w2, b2, w3, b3):
    x = np.asarray(x, dtype=np.float32)
    conv_w = np.asarray(conv_w, dtype=np.float32)
    w1 = np.asarray(w1, dtype=np.float32)
    b1 = np.asarray(b1, dtype=np.float32)
    w2 = np.asarray(w2, dtype=np.float32)
    b2 = np.asarray(b2, dtype=np.float32)
    w3 = np.asarray(w3, dtype=np.float32)
    b3 = np.asarray(b3, dtype=np.float32)

    w1f = _fold_conv_into_w1(conv_w, w1)  # [784, 100]
    # main chunks: feature f = k*128 + p -> [128, 600]
    w1m = np.ascontiguousarray(
        w1f[: 128 * NKC].reshape(NKC, 128, H1).transpose(1, 0, 2)
    ).astype(NP_BF16).reshape(128, NKC * H1)
    w1t = w1f[128 * NKC:].astype(NP_BF16)  # [16, 100]

    blob = np.zeros((128, WBW), np.uint16)
    blob[:, _C_W1M:_C_W1M + NKC * H1] = w1m.view(np.uint16)
    for c in range(2):
        blob[64 * c:64 * c + KT, _C_W1T:_C_W1T + H1] = w1t.view(np.uint16)
    blob[0:H1, _C_W2:_C_W2 + HO] = w2.astype(NP_BF16).view(np.uint16)
    blob[0:HO, _C_W3:_C_W3 + HO] = w3.astype(NP_BF16).view(np.uint16)
    blob[0:H1, _C_B1:_C_B1 + 2] = b1.reshape(H1, 1).view(np.uint16)
    blob[0:HO, _C_B2:_C_B2 + 2] = b2.reshape(HO, 1).view(np.uint16)
    blob[0:HO, _C_B3:_C_B3 + 2] = b3.reshape(HO, 1).view(np.uint16)
    shared = {"wblob": blob.view(NP_BF16)}

    xb = x.astype(NP_BF16)  # cast once, full batch
    in_maps = []
    for core in range(N_CORES):
        xc = xb[core * BC:(core + 1) * BC]  # [8192, 784] bf16
        xct = xc.reshape(NT, TN, NF).transpose(0, 2, 1)  # [NT, NF, TN]
        mains = xct[:, : 128 * NKC].reshape(NT, NKC, 128, TN)
        tails = xct[:, 128 * NKC:]  # [NT, 16, TN]
        xt_all = np.zeros((128, NPAIR, 13, TN), dtype=NP_BF16)
        xt_all[:, :, 0:NKC] = mains[0::2].transpose(2, 0, 1, 3)
        xt_all[:, :, 7:7 + NKC] = mains[1::2].transpose(2, 0, 1, 3)
        xt_all[0:KT, :, NKC] = tails[0::2].transpose(1, 0, 2)
        xt_all[64:64 + KT, :, NKC] = tails[1::2].transpose(1, 0, 2)
        in_maps.append({"xt": xt_all, **shared})
    return in_maps


_NC = None


def _get_nc():
    global _NC
    if _NC is None:
        _NC = _build_nc()
    return _NC


def kernel(x, conv_w, w1, b1, w2, b2, w3, b3):
    in_maps = _prep_in_maps(x, conv_w, w1, b1, w2, b2, w3, b3)
    nc = _get_nc()
    res = run_bass_kernel_spmd(nc, in_maps, core_ids=list(range(N_CORES)))
    out = np.empty((B, HO), dtype=np.float32)
    for i in range(N_CORES):
        out[i * BC:(i + 1) * BC] = res.results[i]["yt"].T
    return out


if __name__ == "__main__":
    rng = np.random.default_rng(0)
    inputs = {
        "x": rng.standard_normal((B, NF), dtype=np.float32),
        "conv_w": np.ones((3, 3), dtype=np.float32),
        "w1": (rng.standard_normal((676, H1)) * 0.04).astype(np.float32),
        "b1": np.zeros(H1, dtype=np.float32),
        "w2": (rng.standard_normal((H1, HO)) * 0.1).astype(np.float32),
        "b2": np.zeros(HO, dtype=np.float32),
        "w3": (rng.standard_normal((HO, HO)) * 0.3).astype(np.float32),
        "b3": np.zeros(HO, dtype=np.float32),
    }
    out = kernel(**inputs)
    print(out.shape, out.dtype)
